# revision 8
# baseline (speedup 1.0000x reference)
"""Chamfer distance kernel for Trainium2, 8 NeuronCores, data-parallel over B.

d[i,j] = ||x_i||^2 + ||y_j||^2 - 2<x_i,y_j> realized as a single 5-dim
matmul contraction: z_i = [x_i, 1, ||x_i||^2], w_j = [-2y_j, ||y_j||^2, 1],
d[i,j] = <z_i, w_j>.  Z/W live as replicated 5-row strips at partitions
{0,32,64,96} so four independent matmuls (tile_position row groups) fill a
[128, 2048] PSUM tile (one i-block x j-quarter-chunk) at 4x PE row use.

dist1 (min over j): tensor_reduce(min) straight off PSUM.
dist2 (min over i): in-place tensor_tensor(min) into a persistent SBUF
accumulator per j-chunk, then a log2 partition fold at the end.
"""

import numpy as np

import concourse.bacc as bacc
import concourse.mybir as mybir
from concourse import tile
from concourse.bass_utils import run_bass_kernel_spmd

F32 = mybir.dt.float32
MIN = mybir.AluOpType.min
ADD = mybir.AluOpType.add
MULT = mybir.AluOpType.mult
AXX = mybir.AxisListType.X

B, N, M, D = 8, 8192, 8192, 3
N_CORES = 8
BIG = 3.0e38


def _build_rep(nc, cp, dp, src_dram, n_pts, scale, sq_then_one, tag):
    """Build the [128, n_pts] replicated 5-row matrix for one input cloud.

    Strip rows p0..p0+4 (p0 in {0,32,64,96}): [scale*x0, scale*x1, scale*x2,
    a, b] where (a, b) = (sq, 1) if sq_then_one else (1, sq).
    """
    nt = n_pts // 128
    rep = cp.tile([128, n_pts], F32, tag=f"rep_{tag}")
    xs = cp.tile([128, nt, 3], F32, tag=f"xs_{tag}")
    nc.gpsimd.dma_start(out=xs[:], in_=src_dram.rearrange("(p t) d -> p t d", p=128))
    xsq = cp.tile([128, nt, 3], F32, tag=f"xsq_{tag}")
    nc.vector.tensor_tensor(xsq[:], xs[:], xs[:], op=MULT)
    sq = cp.tile([128, nt], F32, tag=f"sq_{tag}")
    nc.vector.tensor_reduce(sq[:], xsq[:], axis=AXX, op=ADD)
    sq_d = dp.tile([n_pts], F32, tag=f"sqd_{tag}")
    nc.gpsimd.dma_start(out=sq_d.rearrange("(p t) -> p t", p=128), in_=sq[:])
    xt = cp.tile([128, 3, nt], F32, tag=f"xt_{tag}")
    nc.vector.tensor_scalar_mul(xt.rearrange("p d t -> p t d"), xs[:], scale)
    xt_d = dp.tile([3, n_pts], F32, tag=f"xtd_{tag}")
    nc.gpsimd.dma_start(out=xt_d.rearrange("d (p t) -> p d t", p=128), in_=xt[:])
    ones = cp.tile([1, n_pts], F32, tag=f"ones_{tag}")
    nc.vector.memset(ones[:], 1.0)
    sq_row = sq_d.rearrange("(a q) -> a q", a=1)
    for r in range(4):
        p0 = 32 * r
        nc.gpsimd.dma_start(out=rep[p0 : p0 + 3, :], in_=xt_d[:])
        if sq_then_one:
            nc.gpsimd.dma_start(out=rep[p0 + 3 : p0 + 4, :], in_=sq_row)
            nc.gpsimd.dma_start(out=rep[p0 + 4 : p0 + 5, :], in_=ones[:])
        else:
            nc.gpsimd.dma_start(out=rep[p0 + 3 : p0 + 4, :], in_=ones[:])
            nc.gpsimd.dma_start(out=rep[p0 + 4 : p0 + 5, :], in_=sq_row)
    return rep


def build_chamfer_nc(n=N, m=M, n_cores=N_CORES, iters=1):
    nc = bacc.Bacc("TRN2", num_devices=n_cores)
    x_d = nc.dram_tensor("input1", [n, 3], F32, kind="ExternalInput")
    y_d = nc.dram_tensor("input2", [m, 3], F32, kind="ExternalInput")
    n_blk = n // 128
    chunk = min(2048, m)
    n_chunks = m // chunk
    strip_w = min(512, chunk)
    n_strips = chunk // strip_w
    d1_d = nc.dram_tensor("dist1", [128, n_blk], F32, kind="ExternalOutput")
    d2_d = nc.dram_tensor("dist2", [n_chunks, 128, chunk], F32, kind="ExternalOutput")

    with tile.TileContext(nc) as tc:
        with (
            tc.tile_pool(name="c", bufs=1) as cp,
            tc.tile_pool(name="sc", bufs=3) as sp,
            tc.tile_pool(name="ps", bufs=2, space="PSUM") as pp,
            tc.tile_pool(name="dr", bufs=1, space="DRAM") as dp,
        ):
            # z side from input1 (rows [x,1,sq]); w side from input2 ([-2y,sq,1])
            zrep = _build_rep(nc, cp, dp, x_d, n, 1.0, False, "z")
            wrep = _build_rep(nc, cp, dp, y_d, m, -2.0, True, "w")

            accs = []
            for q in range(n_chunks):
                a = cp.tile([128, chunk], F32, tag=f"acc{q}")
                nc.vector.memset(a[:], BIG)
                accs.append(a)
            d1cols = cp.tile([128, n_blk], F32, tag="d1cols")

            for b in range(n_blk):
                i0 = b * 128
                scr = sp.tile([128, n_chunks], F32, tag="scr")
                for q in range(n_chunks):
                    j0 = q * chunk
                    ps = pp.tile([128, chunk], F32, tag="ps")
                    for s in range(n_strips):
                        p0 = 32 * (s % 4)
                        nc.tensor.matmul(
                            ps[:, s * strip_w : (s + 1) * strip_w],
                            lhsT=zrep[p0 : p0 + 5, i0 : i0 + 128],
                            rhs=wrep[p0 : p0 + 5, j0 + s * strip_w : j0 + (s + 1) * strip_w],
                            tile_position=(p0, 0),
                        )
                    nc.vector.tensor_reduce(scr[:, q : q + 1], ps[:], axis=AXX, op=MIN)
                    nc.vector.tensor_tensor(accs[q][:], accs[q][:], ps[:], op=MIN)
                nc.vector.tensor_reduce(d1cols[:, b : b + 1], scr[:], axis=AXX, op=MIN)

            nc.gpsimd.dma_start(out=d1_d[:], in_=d1cols[:])
            for q in range(n_chunks):
                nc.gpsimd.dma_start(out=d2_d[q], in_=accs[q][:])

    nc.compile()
    return nc


_NC_CACHE = {}


def kernel(input1: np.ndarray, input2: np.ndarray) -> np.ndarray:
    input1 = np.ascontiguousarray(np.asarray(input1, dtype=np.float32))
    input2 = np.ascontiguousarray(np.asarray(input2, dtype=np.float32))
    key = input1.shape
    if key not in _NC_CACHE:
        _NC_CACHE[key] = build_chamfer_nc(
            n=input1.shape[1], m=input2.shape[1], n_cores=N_CORES
        )
    nc = _NC_CACHE[key]
    in_maps = [
        {"input1": input1[b], "input2": input2[b]} for b in range(input1.shape[0])
    ]
    res = run_bass_kernel_spmd(nc, in_maps, core_ids=list(range(N_CORES)))
    s1 = 0.0
    s2 = 0.0
    cnt1 = 0
    cnt2 = 0
    for b in range(input1.shape[0]):
        r = res.results[b]
        d1 = np.asarray(r["dist1"], dtype=np.float64)  # [128, n_blk]
        d2 = np.asarray(r["dist2"], dtype=np.float64).min(axis=1)  # [n_chunks, chunk]
        s1 += d1.sum()
        s2 += d2.sum()
        cnt1 += d1.size
        cnt2 += d2.size
    loss = s1 / cnt1 + s2 / cnt2
    return np.float32(loss)
